# revision 2
# baseline (speedup 1.0000x reference)
"""Trainium2 Bass kernel for EnhancedSelfAttention (GroupNorm + MHSA + proj + residual).

v3: fully software-pipelined, ACT(exp)-bound schedule.

The kernel computes, per core, 2 samples of: GroupNorm(1 group) -> qkv (1x1
conv) -> 8-head self-attention over 1024 pixels -> proj (1x1 conv) -> +x.
Scalar-engine exp over the 8.4M S^T entries per sample is the irreducible
bottleneck (~147us per core); everything else hides underneath it.

Schedule design:
  - One manually-sliced 8-bank PSUM tile: three [128,1024] S^T slots
    (6 banks) + one shared O accumulator (2 banks).
  - Explicit slot-state rotation: after EXP(u) reads slots (ea, eb) with
    "fr" free, ST(u+1) writes (fr, ea) and a per-unit transient (qkv chain
    for sample 1, indicator-broadcast, proj) may take eb. This keeps every
    writer ordered after the reader it displaces while never blocking the
    exp stream.
  - Emission order per unit: EXP(u), ST(u+1), O(u), hooks. Per-engine
    queues are in-order, so this gives ACT back-to-back exps while the PE
    queue never parks on an exp-wait.
  - Sample 0's qkv runs in the prologue; sample 1's 16 qkv chains are
    threaded into pipeline slack at units 0..11.
  - Pair evacuation: one [128,1024] f32->bf16 DVE cast + 4 DMAs (o32 rows,
    denominator rows). Softmax normalization: reciprocal on [4,1024] rows,
    indicator-matmul broadcast, multiply reading PSUM directly. The slow
    reciprocal is spaced 3 units ahead of the broadcast matmuls so they
    never head-of-line block the PE queue.
"""

import sys

import ml_dtypes
import numpy as np

for _p in ("/opt/trn_rl_repo",):
    if _p not in sys.path:
        sys.path.insert(0, _p)

import concourse.bass as bass  # noqa: F401
import concourse.tile as tile
from concourse import bacc, mybir
from concourse.bass_utils import run_bass_kernel_spmd

BF16 = mybir.dt.bfloat16
F32 = mybir.dt.float32
I32 = mybir.dt.int32
AF = mybir.ActivationFunctionType
OP = mybir.AluOpType

B, C, HW = 16, 256, 1024
NH, HD = 8, 32
NCORES = 8
SPC = B // NCORES  # samples per core
EPS = 1e-5
SCALE = float(HD) ** -0.5

_CACHE: dict = {}

_IND4 = np.zeros((128, 128), np.float32)
for _i in range(4):
    _IND4[_i, 32 * _i : 32 * _i + 32] = 1.0


def _emit_gn_stats(nc, tp, x_sb):
    """GroupNorm phase 1: per-partition stats (DVE only)."""
    stat6 = tp.tile([128, 4, 6], F32, tag="stat6")
    for i in range(4):
        nc.vector.bn_stats(
            out=stat6[:, i, :], in_=x_sb[:, i // 2, 512 * (i % 2) : 512 * (i % 2) + 512]
        )
    mv = tp.tile([128, 2], F32, tag="mv")
    nc.vector.bn_aggr(out=mv, in_=stat6)
    st2 = tp.tile([128, 2], F32, tag="st2")
    nc.vector.tensor_copy(out=st2[:, 0:1], in_=mv[:, 0:1])
    nc.vector.scalar_tensor_tensor(
        out=st2[:, 1:2],
        in0=mv[:, 0:1],
        scalar=mv[:, 0:1],
        in1=mv[:, 1:2],
        op0=OP.mult,
        op1=OP.add,
    )
    return st2


def _emit_gn(nc, tp, st2, ps_g, ps_b, x_sb, xn_sb, gnw_sb, gnb_sb, ones_col, ones_row):
    """GroupNorm phase 2: reduction, rsqrt, affine."""
    nc.tensor.matmul(out=ps_g, lhsT=ones_col, rhs=st2, start=True, stop=True)
    sc = tp.tile([1, 8], F32, tag="sc")
    nc.vector.tensor_scalar_mul(out=sc[:, 0:2], in0=ps_g, scalar1=1.0 / 128.0)
    nc.vector.tensor_mul(out=sc[:, 2:3], in0=sc[:, 0:1], in1=sc[:, 0:1])
    nc.vector.tensor_sub(out=sc[:, 3:4], in0=sc[:, 1:2], in1=sc[:, 2:3])
    nc.vector.tensor_scalar_add(out=sc[:, 4:5], in0=sc[:, 3:4], scalar1=EPS)
    vep = sc[:, 4:5]
    yt = tp.tile([1, 8], F32, tag="yt")
    nc.vector.tensor_scalar(
        out=yt[:, 0:1].bitcast(I32),
        in0=vep.bitcast(I32),
        scalar1=1,
        scalar2=None,
        op0=OP.logical_shift_right,
    )
    nc.vector.tensor_scalar(
        out=yt[:, 1:2].bitcast(I32),
        in0=yt[:, 0:1].bitcast(I32),
        scalar1=-1,
        scalar2=0x5F3759DF,
        op0=OP.mult,
        op1=OP.add,
    )
    y = yt[:, 1:2]
    for it in range(3):
        t0 = yt[:, 2 + it : 3 + it] if it < 2 else yt[:, 2 + (it % 2) : 3 + (it % 2)]
        nc.vector.tensor_mul(out=t0, in0=vep, in1=y)
        nc.vector.tensor_mul(out=t0, in0=t0, in1=y)
        nc.vector.tensor_scalar(
            out=t0, in0=t0, scalar1=-0.5, scalar2=1.5, op0=OP.mult, op1=OP.add
        )
        ynew = yt[:, 4 + (it % 2) : 5 + (it % 2)]
        nc.vector.tensor_mul(out=ynew, in0=y, in1=t0)
        y = ynew
    fin = tp.tile([1, 2], F32, tag="fin")
    nc.vector.tensor_scalar_mul(out=fin[:, 0:1], in0=sc[:, 0:1], scalar1=-1.0)
    nc.vector.tensor_copy(out=fin[:, 1:2], in_=y)
    nc.tensor.matmul(out=ps_b, lhsT=ones_row, rhs=fin, start=True, stop=True)
    bc = tp.tile([128, 2], F32, tag="bc")
    nc.vector.tensor_copy(out=bc, in_=ps_b)
    a_sb = tp.tile([128, 2], F32, tag="asb")
    nc.vector.tensor_scalar_mul(out=a_sb, in0=gnw_sb, scalar1=bc[:, 1:2])
    b_sb = tp.tile([128, 2], F32, tag="bsb")
    nc.vector.scalar_tensor_tensor(
        out=b_sb, in0=a_sb, scalar=bc[:, 0:1], in1=gnb_sb, op0=OP.mult, op1=OP.add
    )
    for k in range(2):
        nc.vector.tensor_scalar(
            out=xn_sb[:, k, :],
            in0=x_sb[:, k, :],
            scalar1=a_sb[:, k : k + 1],
            scalar2=b_sb[:, k : k + 1],
            op0=OP.mult,
            op1=OP.add,
        )


def _build():
    nc = bacc.Bacc("TRN2", target_bir_lowering=False, debug=False)
    x_d = nc.dram_tensor("x", [SPC, C, HW], F32, kind="ExternalInput").ap()
    qkvwT_d = nc.dram_tensor("qkv_wT", [C, 3 * C], BF16, kind="ExternalInput").ap()
    qbqk_d = nc.dram_tensor("qkv_b_qk", [4, 128], F32, kind="ExternalInput").ap()
    qbv_d = nc.dram_tensor("qkv_b_v", [1, C], F32, kind="ExternalInput").ap()
    pwT_d = nc.dram_tensor("proj_wT", [C, C], BF16, kind="ExternalInput").ap()
    pb_d = nc.dram_tensor("proj_b", [2, 128], F32, kind="ExternalInput").ap()
    gnw_d = nc.dram_tensor("gn_w", [2, 128], F32, kind="ExternalInput").ap()
    gnb_d = nc.dram_tensor("gn_b", [2, 128], F32, kind="ExternalInput").ap()
    ind4_d = nc.dram_tensor("ind4", [128, 128], BF16, kind="ExternalInput").ap()
    out_d = nc.dram_tensor("out", [SPC, C, HW], F32, kind="ExternalOutput").ap()

    with tile.TileContext(nc) as tc:
        _emit(
            nc, tc, x_d, qkvwT_d, qbqk_d, qbv_d, pwT_d, pb_d, gnw_d, gnb_d, ind4_d,
            out_d,
        )
    nc.compile()
    return nc


def _emit(
    nc, tc, x_d, qkvwT_d, qbqk_d, qbv_d, pwT_d, pb_d, gnw_d, gnb_d, ind4_d, out_d
):
    from contextlib import ExitStack

    with ExitStack() as ctx:
        singles = ctx.enter_context(tc.tile_pool(name="singles", bufs=1))
        samp = ctx.enter_context(tc.tile_pool(name="samp", bufs=2))
        tp = ctx.enter_context(tc.tile_pool(name="small", bufs=3))
        e_pool = ctx.enter_context(tc.tile_pool(name="epool", bufs=4))
        stagp = ctx.enter_context(tc.tile_pool(name="stagp", bufs=2))
        psp = ctx.enter_context(tc.tile_pool(name="psp", bufs=1, space="PSUM"))

        # ---- the whole PSUM, manually sliced ----
        PS = psp.tile([128, 4096], F32, name="PS")
        OPS = PS[:, 3072:4096]  # O accumulator (2 banks), shared by all pairs

        def slot_ap(k):
            return PS[:, 1024 * k : 1024 * k + 1024]

        _pc = [0]  # prologue slot counter

        def pslot():
            k = _pc[0] % 3
            _pc[0] += 1
            return slot_ap(k)

        # ---- inputs ----
        x_tiles = []
        for s in range(SPC):
            x_sb = samp.tile([128, 2, HW], F32, name="x_sb", tag="x")
            nc.sync.dma_start(
                out=x_sb, in_=x_d[s].rearrange("(k p) n -> p k n", p=128)
            )
            x_tiles.append(x_sb)

        qkvwT = singles.tile([128, 2, 3 * C], BF16)
        nc.sync.dma_start(out=qkvwT, in_=qkvwT_d.rearrange("(k p) o -> p k o", p=128))
        pwT = singles.tile([128, 2, C], BF16)
        nc.sync.dma_start(out=pwT, in_=pwT_d.rearrange("(k p) o -> p k o", p=128))
        qb_sb = singles.tile([128, 4], F32)
        nc.sync.dma_start(out=qb_sb, in_=qbqk_d.rearrange("t p -> p t"))
        pb_sb = singles.tile([128, 2], F32)
        nc.sync.dma_start(out=pb_sb, in_=pb_d.rearrange("t p -> p t"))
        gnw_sb = singles.tile([128, 2], F32)
        nc.sync.dma_start(out=gnw_sb, in_=gnw_d.rearrange("t p -> p t"))
        gnb_sb = singles.tile([128, 2], F32)
        nc.sync.dma_start(out=gnb_sb, in_=gnb_d.rearrange("t p -> p t"))
        qbv_sb = singles.tile([1, C], F32)
        nc.sync.dma_start(out=qbv_sb, in_=qbv_d)
        ind4_sb = singles.tile([128, 128], BF16)
        nc.sync.dma_start(out=ind4_sb, in_=ind4_d)

        zeros_col = singles.tile([128, 1], F32)
        nc.vector.memset(zeros_col, 0.0)
        ones_col = singles.tile([128, 1], F32)
        nc.vector.memset(ones_col, 1.0)
        ones_row = singles.tile([1, 128], F32)
        nc.vector.memset(ones_row, 1.0)
        kpad = []
        for i in range(4):
            kp = singles.tile([128, HW], BF16, name=f"kpad{i}")
            nc.vector.tensor_copy(out=kp, in_=zeros_col.to_broadcast([128, HW]))
            kpad.append(kp)
        # persistent normalization tiles: rows 4..127 zeroed once
        rs_raw = []
        rsi_t = []
        for cpar in range(2):
            rr = singles.tile([4, HW], BF16, name=f"rsraw{cpar}")
            rs_raw.append(rr)
            rt = singles.tile([128, HW], BF16, name=f"rsit{cpar}")
            nc.vector.tensor_copy(out=rt, in_=zeros_col.to_broadcast([128, HW]))
            rsi_t.append(rt)

        # dummy exp: pulls the ~2.7us ACT table load off the critical path
        dummy_e = tp.tile([1, 8], F32, name="dummy_e", tag="de")
        nc.scalar.activation(out=dummy_e, in_=ones_row[:, 0:8], func=AF.Exp, scale=0.01)
        # bf16 matmul burst: pre-warms the HAM clock gate during GN
        db = singles.tile([128, 512], BF16)
        nc.vector.memset(db, 0.5)

        def warm_mm(n=1):
            for _ in range(n):
                nc.tensor.matmul(
                    out=OPS[0:64, 0:512],
                    lhsT=db[:, 0:64],
                    rhs=db,
                    start=True,
                    stop=True,
                    skip_group_check=True,
                )

        warm_mm(10)

        # ---- GroupNorm: sample 0 fully; sample 1 stats now, rest after
        # sample 0's qkv so its tiny matmuls don't block qkv in the PE queue
        xn_tiles = [
            samp.tile([128, 2, HW], BF16, name="xn_sb", tag="xn") for s in range(SPC)
        ]
        st2_0 = _emit_gn_stats(nc, tp, x_tiles[0])
        slot = pslot()
        _emit_gn(
            nc, tp, st2_0, slot[0:1, 0:2], slot[:, 512:514], x_tiles[0],
            xn_tiles[0], gnw_sb, gnb_sb, ones_col, ones_row,
        )
        st2_1 = _emit_gn_stats(nc, tp, x_tiles[1])

        # v-part bias broadcast along partitions: [128, 256]
        slot = pslot()
        nc.tensor.matmul(
            out=slot[:, 0:C], lhsT=ones_row, rhs=qbv_sb, start=True, stop=True
        )
        vb_bc = singles.tile([128, C], F32)
        nc.vector.tensor_copy(out=vb_bc, in_=slot[:, 0:C])

        # ---- qkv chains (per-chain so sample 1's can thread into the pipeline)
        qk_tiles = [
            samp.tile([128, 4, HW], BF16, name="qk_sb", tag="qk") for s in range(SPC)
        ]
        vn_tiles = []
        for s in range(SPC):
            vn_sb = samp.tile([128, 8, NH, HD + 1], BF16, name="vn_sb", tag="vn")
            nc.vector.tensor_copy(
                out=vn_sb[:, :, :, HD : HD + 1],
                in_=ones_col.to_broadcast([128, 8, NH, 1]),
            )
            vn_tiles.append(vn_sb)

        def qk_chain(s, mt, slot):
            for hf in range(2):
                for kc in range(2):
                    nc.tensor.matmul(
                        out=slot[:, 512 * hf : 512 * hf + 512],
                        lhsT=qkvwT[:, kc, 128 * mt : 128 * mt + 128],
                        rhs=xn_tiles[s][:, kc, 512 * hf : 512 * hf + 512],
                        start=(kc == 0),
                        stop=(kc == 1),
                        skip_group_check=True,
                    )
            nc.vector.tensor_scalar_add(
                out=qk_tiles[s][:, mt, :], in0=slot, scalar1=qb_sb[:, mt : mt + 1]
            )

        def v_chain(s, jj, slot):
            for kc in range(2):
                nc.tensor.matmul(
                    out=slot[:, 0:C],
                    lhsT=xn_tiles[s][:, kc, 128 * jj : 128 * jj + 128],
                    rhs=qkvwT[:, kc, 2 * C : 3 * C],
                    start=(kc == 0),
                    stop=(kc == 1),
                    skip_group_check=True,
                )
            nc.vector.tensor_add(
                out=vn_tiles[s][:, jj, :, 0:HD],
                in0=slot[:, 0:C].rearrange("p (h d) -> p h d", h=NH),
                in1=vb_bc.rearrange("p (h d) -> p h d", h=NH),
            )

        # sample 0 qkv in the prologue (k chunk mt=2 first -> kpad DMA early)
        for mt in (2, 0, 3, 1):
            qk_chain(0, mt, pslot())
        for jj in range(8):
            v_chain(0, jj, pslot())
        # sample 1 GroupNorm phase 2 (its DVE chain overlaps qkv's matmuls)
        slot = pslot()
        _emit_gn(
            nc, tp, st2_1, slot[0:1, 0:2], slot[:, 512:514], x_tiles[1],
            xn_tiles[1], gnw_sb, gnb_sb, ones_col, ones_row,
        )

        o32_tiles = [
            samp.tile([128, 2, HW], BF16, name="o32_sb", tag="o32") for s in range(SPC)
        ]
        out_tiles = [
            samp.tile([128, 2, HW], F32, name="out_sb", tag="outsb") for s in range(SPC)
        ]

        # ---- attention pipeline ----
        st_slots = {}
        e_tiles = {}
        bc_ps = {}

        def kpad_dma(P):
            s, pr = P // 4, P % 4
            for h in (2 * pr, 2 * pr + 1):
                qb = 32 * (h % 4)
                mk = 2 + h // 4
                nc.sync.dma_start(
                    out=kpad[h % 4][qb : qb + 32, :],
                    in_=qk_tiles[s][qb : qb + 32, mk, :],
                )

        def emit_ST(u, sa, sb):
            s, pr = u // 32, (u % 32) // 8
            j = u % 8
            for h, sk in ((2 * pr, sa), (2 * pr + 1, sb)):
                mq = h // 4
                slot = slot_ap(sk)
                for hf in range(2):
                    nc.tensor.matmul(
                        out=slot[:, 512 * hf : 512 * hf + 512],
                        lhsT=kpad[h % 4][:, 128 * j : 128 * j + 128],
                        rhs=qk_tiles[s][:, mq, 512 * hf : 512 * hf + 512],
                        start=True,
                        stop=True,
                        skip_group_check=True,
                    )
            st_slots[u] = (sa, sb)

        def emit_EXP(u):
            sa, sb = st_slots[u]
            e = e_pool.tile([128, 2048], BF16, name="e", tag="e")
            nc.scalar.activation(
                out=e[:, 0:1024], in_=slot_ap(sa), func=AF.Exp, scale=SCALE
            )
            nc.scalar.activation(
                out=e[:, 1024:2048], in_=slot_ap(sb), func=AF.Exp, scale=SCALE
            )
            e_tiles[u] = e

        def emit_O(u):
            s, pr = u // 32, (u % 32) // 8
            j = u % 8
            e = e_tiles[u]
            for hf in range(2):
                for t in range(2):
                    cg = 64 * t
                    nc.tensor.matmul(
                        out=OPS[cg : cg + 33, 512 * hf : 512 * hf + 512],
                        lhsT=vn_tiles[s][:, j, 2 * pr + t, :],
                        rhs=e[:, 1024 * t + 512 * hf : 1024 * t + 512 * hf + 512],
                        start=(j == 0),
                        stop=(j == 7),
                        tile_position=(0, cg),
                        skip_group_check=True,
                    )

        last_stag = [None]

        def emit_evac(P):
            s, pr = P // 4, P % 4
            chunk = pr // 2
            qA = 64 * (pr % 2)
            stag = stagp.tile([128, HW], BF16, name="stag", tag="stag")
            last_stag[0] = stag
            nc.vector.tensor_copy(out=stag, in_=OPS)
            nc.sync.dma_start(
                out=o32_tiles[s][qA : qA + 32, chunk, :], in_=stag[0:32, :]
            )
            nc.sync.dma_start(
                out=o32_tiles[s][qA + 32 : qA + 64, chunk, :], in_=stag[64:96, :]
            )
            r0 = (2 * pr) % 4
            nc.sync.dma_start(out=rs_raw[chunk][r0 : r0 + 1, :], in_=stag[32:33, :])
            nc.sync.dma_start(
                out=rs_raw[chunk][r0 + 1 : r0 + 2, :], in_=stag[96:97, :]
            )

        def emit_recip(chunk):
            with nc.allow_low_precision(reason="softmax denominators"):
                for hf in range(2):
                    nc.vector.reciprocal(
                        out=rsi_t[chunk][0:4, 512 * hf : 512 * hf + 512],
                        in_=rs_raw[chunk][:, 512 * hf : 512 * hf + 512],
                    )

        def emit_bc(s, chunk, slot):
            for hf in range(2):
                nc.tensor.matmul(
                    out=slot[:, 512 * hf : 512 * hf + 512],
                    lhsT=ind4_sb,
                    rhs=rsi_t[chunk][:, 512 * hf : 512 * hf + 512],
                    start=True,
                    stop=True,
                    skip_group_check=True,
                )
            bc_ps[(s, chunk)] = slot

        def emit_apply(s, chunk):
            nc.vector.tensor_mul(
                out=o32_tiles[s][:, chunk, :],
                in0=o32_tiles[s][:, chunk, :],
                in1=bc_ps[(s, chunk)],
            )

        def emit_proj(s, mt, slot):
            for hf in range(2):
                for kc in range(2):
                    nc.tensor.matmul(
                        out=slot[:, 512 * hf : 512 * hf + 512],
                        lhsT=pwT[:, kc, 128 * mt : 128 * mt + 128],
                        rhs=o32_tiles[s][:, kc, 512 * hf : 512 * hf + 512],
                        start=(kc == 0),
                        stop=(kc == 1),
                        skip_group_check=True,
                    )
            nc.vector.scalar_tensor_tensor(
                out=out_tiles[s][:, mt, :],
                in0=slot,
                scalar=pb_sb[:, mt : mt + 1],
                in1=x_tiles[s][:, mt, :],
                op0=OP.add,
                op1=OP.add,
            )
            nc.sync.dma_start(
                out=out_d[s].rearrange("(k p) n -> p k n", p=128)[:, mt, :],
                in_=out_tiles[s][:, mt, :],
            )

        # hooks after O(u): "t" entries get the transient slot (<=1 per unit)
        hooks = {}

        def add_hook(u, kind, fn):
            hooks.setdefault(u, []).append((kind, fn))

        # sample-1 qkv threaded into units 0..11
        for i, mt in enumerate((2, 0, 3, 1)):
            add_hook(i, "t", lambda sl, mt=mt: qk_chain(1, mt, sl))
        for jj in range(8):
            add_hook(4 + jj, "t", lambda sl, jj=jj: v_chain(1, jj, sl))
        # normalization chains, spaced so the reciprocal never blocks PE
        # bc+apply share one hook: apply must be emitted before the next
        # unit's ST reuses the bc slot (readers must precede the next writer
        # in program order for the dependency to point the right way)
        add_hook(16, "d", lambda: emit_recip(0))
        add_hook(22, "t", lambda sl: (emit_bc(0, 0, sl), emit_apply(0, 0)))
        add_hook(32, "d", lambda: emit_recip(1))
        add_hook(38, "t", lambda sl: (emit_bc(0, 1, sl), emit_apply(0, 1)))
        add_hook(39, "t", lambda sl: emit_proj(0, 0, sl))
        add_hook(40, "t", lambda sl: emit_proj(0, 1, sl))
        add_hook(48, "d", lambda: emit_recip(0))
        add_hook(54, "t", lambda sl: (emit_bc(1, 0, sl), emit_apply(1, 0)))

        kpad_dma(0)
        ea, eb, fr = 0, 1, 2
        emit_ST(0, ea, eb)
        for u in range(64):
            emit_EXP(u)
            if u < 63:
                emit_ST(u + 1, fr, ea)
            emit_O(u)
            s, pr = u // 32, (u % 32) // 8
            j = u % 8
            P = 4 * s + pr
            if j == 0 and P + 1 < 8:
                kpad_dma(P + 1)
            if j == 7:
                emit_evac(P)
            for kind, fn in hooks.get(u, ()):
                if kind == "t":
                    fn(slot_ap(eb))
                else:
                    fn()
            ea, eb, fr = fr, ea, eb

        # tail: sample 1 chunk 1 normalize + proj, hf-pipelined.
        # Keep-warm matmuls are CHAINED on tail data so they spread across the
        # tail instead of bunching up at its start.
        def warm_on(rhs_ap):
            nc.tensor.matmul(
                out=OPS[0:64, 0:512],
                lhsT=db[:, 0:64],
                rhs=rhs_ap,
                start=True,
                stop=True,
                skip_group_check=True,
            )

        stag7 = last_stag[0]
        bcs = slot_ap(fr)
        warm_on(stag7[:, 0:512])
        with nc.allow_low_precision(reason="softmax denominators"):
            nc.vector.reciprocal(
                out=rsi_t[1][0:4, 0:512], in_=rs_raw[1][:, 0:512]
            )
        warm_on(rsi_t[1][:, 0:512])
        nc.tensor.matmul(
            out=bcs[:, 0:512], lhsT=ind4_sb, rhs=rsi_t[1][:, 0:512],
            start=True, stop=True, skip_group_check=True,
        )
        nc.vector.tensor_mul(
            out=o32_tiles[1][:, 1, 0:512],
            in0=o32_tiles[1][:, 1, 0:512],
            in1=bcs[:, 0:512],
        )
        with nc.allow_low_precision(reason="softmax denominators"):
            nc.vector.reciprocal(
                out=rsi_t[1][0:4, 512:1024], in_=rs_raw[1][:, 512:1024]
            )

        def proj_hf(s, hf, slot):
            for mt in range(2):
                for kc in range(2):
                    nc.tensor.matmul(
                        out=slot[:, 512 * mt : 512 * mt + 512],
                        lhsT=pwT[:, kc, 128 * mt : 128 * mt + 128],
                        rhs=o32_tiles[s][:, kc, 512 * hf : 512 * hf + 512],
                        start=(kc == 0),
                        stop=(kc == 1),
                        skip_group_check=True,
                    )
            for mt in range(2):
                nc.vector.scalar_tensor_tensor(
                    out=out_tiles[s][:, mt, 512 * hf : 512 * hf + 512],
                    in0=slot[:, 512 * mt : 512 * mt + 512],
                    scalar=pb_sb[:, mt : mt + 1],
                    in1=x_tiles[s][:, mt, 512 * hf : 512 * hf + 512],
                    op0=OP.add,
                    op1=OP.add,
                )
                nc.sync.dma_start(
                    out=out_d[s].rearrange("(k p) n -> p k n", p=128)[
                        :, mt, 512 * hf : 512 * hf + 512
                    ],
                    in_=out_tiles[s][:, mt, 512 * hf : 512 * hf + 512],
                )

        proj_hf(1, 0, slot_ap(ea))
        warm_on(rsi_t[1][:, 512:1024])
        nc.tensor.matmul(
            out=bcs[:, 512:1024], lhsT=ind4_sb, rhs=rsi_t[1][:, 512:1024],
            start=True, stop=True, skip_group_check=True,
        )
        nc.vector.tensor_mul(
            out=o32_tiles[1][:, 1, 512:1024],
            in0=o32_tiles[1][:, 1, 512:1024],
            in1=bcs[:, 512:1024],
        )
        proj_hf(1, 1, slot_ap(eb))


def _get_nc():
    if "nc" not in _CACHE:
        _CACHE["nc"] = _build()
    return _CACHE["nc"]


def kernel(x, gn_w, gn_b, qkv_w, qkv_b, proj_w, proj_b, **_ignored):
    nc = _get_nc()
    x = np.asarray(x, dtype=np.float32).reshape(B, C, HW)
    qkv_wT = np.ascontiguousarray(
        np.asarray(qkv_w, np.float32).T.astype(ml_dtypes.bfloat16)
    )
    proj_wT = np.ascontiguousarray(
        np.asarray(proj_w, np.float32).T.astype(ml_dtypes.bfloat16)
    )
    qkv_b = np.asarray(qkv_b, np.float32)
    shared = {
        "qkv_wT": qkv_wT,
        "qkv_b_qk": np.ascontiguousarray(qkv_b[: 2 * C].reshape(4, 128)),
        "qkv_b_v": np.ascontiguousarray(qkv_b[2 * C :].reshape(1, C)),
        "proj_wT": proj_wT,
        "proj_b": np.ascontiguousarray(np.asarray(proj_b, np.float32).reshape(2, 128)),
        "gn_w": np.ascontiguousarray(np.asarray(gn_w, np.float32).reshape(2, 128)),
        "gn_b": np.ascontiguousarray(np.asarray(gn_b, np.float32).reshape(2, 128)),
        "ind4": _IND4.astype(ml_dtypes.bfloat16),
    }
    in_maps = [
        {"x": np.ascontiguousarray(x[i * SPC : (i + 1) * SPC]), **shared}
        for i in range(NCORES)
    ]
    br = run_bass_kernel_spmd(nc, in_maps, core_ids=list(range(NCORES)))
    out = np.concatenate([r["out"] for r in br.results], axis=0)
    return out.reshape(B, C, 32, 32)
